# revision 1
# baseline (speedup 1.0000x reference)
"""PhiHarmonicAttention (B=1, S=2048, D=2048, H=16, Dh=128) on 8 Trainium2 cores.

Sharding: tensor-parallel over heads — 2 heads per core.
  - Wq/Wk/Wv column-sliced (256 cols per core), Wo row-sliced (256 rows).
  - Each core computes q^T/k^T (RoPE'd) + v for its 2 heads, causal
    softmax(QK^T)V in transposed layout, and a partial x-out product with its
    Wo slice. Host sums the 8 partials (TP row-parallel reduction).

All matmuls run in float32r (TF32-like, full PE rate at N>=256,
~1e-4 relative rounding). Scores are computed without max subtraction
(valid: scores ~ N(0,1), |scores| < ~6, exp is safe in fp32).

Pipeline: per 512-wide seq chunk j — projections A(j), then attention
B(h0,j), B(h1,j), then output projection C(j). Causality means B(*,j) only
needs A(0..j), so PE never starves at stage boundaries.
"""
import numpy as np
from contextlib import ExitStack, nullcontext

import concourse.bass as bass
import concourse.tile as tile
from concourse import bacc, mybir
from concourse.bass_utils import run_bass_kernel_spmd

S = 2048
D = 2048
H = 16
DH = 128
NCORES = 8
HPC = H // NCORES          # heads per core = 2
CW = HPC * DH              # weight col-slice per core = 256
NO = D // 128              # contraction chunks = 16
NJ = S // 512              # 512-wide table chunks = 4 (rope tables)
# seq chunks (start, width): smaller first chunks so attention starts before
# the full 12MB of weights+xt(0) lands (startup is DMA-bound)
CHUNKS = [(0, 512), (512, 512), (1024, 512), (1536, 512)]
NB = S // 128              # 128-wide seq blocks = 16
SCALE = float(1.0 / np.sqrt(np.float32(DH)))

ROT_FACTOR = (1.0 + 5.0 ** 0.5) / 2.0 - 1.0
ROPE_BASE = 10000.0

F32 = mybir.dt.float32
F32R = mybir.dt.float32r


def _build_nc(reps=1, stages="ABC"):
    nc = bacc.Bacc("TRN2", target_bir_lowering=False, debug=False, num_devices=NCORES)

    xt_d = nc.dram_tensor("xt", [D, S], F32R, kind="ExternalInput").ap()
    wq_d = nc.dram_tensor("wq", [D, CW], F32R, kind="ExternalInput").ap()
    wk_d = nc.dram_tensor("wk", [D, CW], F32R, kind="ExternalInput").ap()
    wv_d = nc.dram_tensor("wv", [D, CW], F32R, kind="ExternalInput").ap()
    wo_d = nc.dram_tensor("wo", [CW, D], F32R, kind="ExternalInput").ap()
    rcu_d = nc.dram_tensor("ropecu", [DH, 512], F32, kind="ExternalInput").ap()
    rsu_d = nc.dram_tensor("ropesu", [DH, 512], F32, kind="ExternalInput").ap()
    rc512_d = nc.dram_tensor("ropec512", [DH, NJ], F32, kind="ExternalInput").ap()
    rs512_d = nc.dram_tensor("ropes512", [DH, NJ], F32, kind="ExternalInput").ap()
    onc_d = nc.dram_tensor("onescol", [128, 1], F32R, kind="ExternalInput").ap()
    out_d = nc.dram_tensor("out", [S, D], mybir.dt.float16, kind="ExternalOutput").ap()

    with ExitStack() as ctx:
        tc = ctx.enter_context(tile.TileContext(nc))
        consts = ctx.enter_context(tc.tile_pool(name="consts", bufs=1))
        persist = ctx.enter_context(tc.tile_pool(name="persist", bufs=1))
        xw = ctx.enter_context(tc.tile_pool(name="xw", bufs=11))
        ptp = ctx.enter_context(tc.tile_pool(name="ptp", bufs=4))
        work = ctx.enter_context(tc.tile_pool(name="work", bufs=2))
        outp = ctx.enter_context(tc.tile_pool(name="outp", bufs=6))
        ps = ctx.enter_context(tc.tile_pool(name="ps", bufs=8, space="PSUM"))

        # ---- constants (split DMAs so o=0 weight chunks land first) ----
        wq_s = consts.tile([128, NO, CW], F32R, tag="wq")
        wk_s = consts.tile([128, NO, CW], F32R, tag="wk")
        wv_s = consts.tile([128, NO, CW], F32R, tag="wv")
        wo_s = consts.tile([128, HPC, D], F32R, tag="wo")
        rc = consts.tile([DH, S], F32, tag="rc")
        rs = consts.tile([DH, S], F32, tag="rs")
        msk = consts.tile([128, 4, 512], mybir.dt.bfloat16, tag="msk")
        onc = consts.tile([128, 1], F32R, tag="onc")
        rcu = consts.tile([DH, 512], F32, tag="rcu")
        rsu = consts.tile([DH, 512], F32, tag="rsu")
        rc512 = consts.tile([DH, NJ], F32, tag="rc512")
        rs512 = consts.tile([DH, NJ], F32, tag="rs512")
        for o in range(NO):
            nc.scalar.dma_start(wv_s[:, o, :], wv_d[128 * o:128 * (o + 1), :])
            nc.scalar.dma_start(wq_s[:, o, :], wq_d[128 * o:128 * (o + 1), :])
            nc.scalar.dma_start(wk_s[:, o, :], wk_d[128 * o:128 * (o + 1), :])
            if o == 0:
                nc.scalar.dma_start(rcu[:], rcu_d)
                nc.scalar.dma_start(rsu[:], rsu_d)
                nc.scalar.dma_start(rc512[:], rc512_d)
                nc.scalar.dma_start(rs512[:], rs512_d)
        for j in range(NJ):
            sl = slice(512 * j, 512 * (j + 1))
            tm = work.tile([128, 512], F32, tag="t1")
            nc.vector.tensor_scalar_mul(tm[:], rsu[:], rs512[:, j:j + 1])
            nc.vector.scalar_tensor_tensor(
                rc[:, sl], rcu[:], rc512[:, j:j + 1], tm[:],
                mybir.AluOpType.mult, mybir.AluOpType.subtract,
            )
            tm2 = work.tile([128, 512], F32, tag="tsw")
            nc.vector.tensor_scalar_mul(tm2[:], rcu[:], rs512[:, j:j + 1])
            nc.vector.scalar_tensor_tensor(
                rs[:, sl], rsu[:], rc512[:, j:j + 1], tm2[:],
                mybir.AluOpType.mult, mybir.AluOpType.add,
            )
        iot = work.tile([128, 512], F32, tag="t1")
        nc.gpsimd.iota(
            iot[:], pattern=[[1, 512]], base=0, channel_multiplier=-1,
            allow_small_or_imprecise_dtypes=True,
        )
        for r in range(4):
            nc.vector.tensor_scalar(
                msk[:, r, :], iot[:], float(128 * r), None,
                mybir.AluOpType.is_ge,
            )
        nc.scalar.dma_start(onc[:], onc_d)
        nc.scalar.dma_start(wo_s[:], wo_d.rearrange("(h p) n -> p h n", p=128))

        rep_ctx = (
            tc.For_i(
                0, reps, 1,
                hint_engines=tuple(
                    getattr(mybir.EngineType, e)
                    for e in ("PE", "DVE", "Activation", "SP", "Pool")
                ),
            )
            if reps > 1 else nullcontext()
        )
        ctx.enter_context(rep_ctx)

        # ---- persistent per-head tensors ----
        qT = [persist.tile([DH, S], F32R, tag=f"qT{h}", name=f"qT{h}")
              for h in range(HPC)]
        kT = [persist.tile([DH, S], F32R, tag=f"kT{h}", name=f"kT{h}")
              for h in range(HPC)]
        v_sb = persist.tile([128, NB, CW], F32R, tag="v")
        aT = [persist.tile([DH, S], F32R, tag=f"aT{h}", name=f"aT{h}")
              for h in range(HPC)]

        def rope_apply(psum, dst_slice, s0, w):
            cs = rc[:, s0:s0 + w]
            sn = rs[:, s0:s0 + w]
            raw = work.tile([128, 512], F32, tag="raw")
            nc.scalar.copy(raw[:, :w], psum[:])      # frees the PSUM bank fast
            t1 = work.tile([128, 512], F32, tag="t1")
            nc.vector.tensor_mul(t1[:, :w], raw[:, :w], cs)
            tsw = work.tile([128, 512], F32, tag="tsw")
            nc.vector.tensor_copy(tsw[0:64, :w], raw[64:128, :w])
            nc.vector.tensor_copy(tsw[64:128, :w], raw[0:64, :w])
            nc.vector.tensor_mul(tsw[:, :w], tsw[:, :w], sn)
            nc.vector.tensor_add(dst_slice, t1[:, :w], tsw[:, :w])

        def stage_a(ci):
            s0, w = CHUNKS[ci]
            nblk = w // 128
            pq = [ps.tile([128, w], F32, tag="ps", name=f"pq{ci}_{i}")
                  for i in range(HPC)]
            pk = [ps.tile([128, w], F32, tag="ps", name=f"pk{ci}_{i}")
                  for i in range(HPC)]
            pv = [ps.tile([128, CW], F32, tag="ps", name=f"pv{ci}_{i}")
                  for i in range(nblk)]
            for o in range(NO):
                xt_t = xw.tile([128, w], F32R, tag="xt")
                nc.sync.dma_start(
                    xt_t[:], xt_d[128 * o:128 * (o + 1), s0:s0 + w]
                )
                st = dict(start=(o == 0), stop=(o == NO - 1))
                for m4 in range(nblk):
                    nc.tensor.matmul(
                        pv[m4][:], xt_t[:, 128 * m4:128 * (m4 + 1)],
                        wv_s[:, o, :], **st
                    )
                for h in range(HPC):
                    nc.tensor.matmul(
                        pq[h][:], wq_s[:, o, 128 * h:128 * (h + 1)],
                        xt_t[:], **st
                    )
                    nc.tensor.matmul(
                        pk[h][:], wk_s[:, o, 128 * h:128 * (h + 1)],
                        xt_t[:], **st
                    )
            for m4 in range(nblk):
                nc.vector.tensor_copy(v_sb[:, s0 // 128 + m4, :], pv[m4][:])
            rope_apply(pq[0], qT[0][:, s0:s0 + w], s0, w)
            rope_apply(pk[0], kT[0][:, s0:s0 + w], s0, w)
            rope_apply(pq[1], qT[1][:, s0:s0 + w], s0, w)
            rope_apply(pk[1], kT[1][:, s0:s0 + w], s0, w)

        def stage_b(h, ci):
            s0, w = CHUNKS[ci]
            nb = (s0 + w) // 128
            sb0 = s0 // 128
            po = ps.tile([128, w], F32, tag="ps", name=f"po{h}_{ci}")
            pd = ps.tile([1, w], F32, tag="ps", name=f"pd{h}_{ci}")
            for b in range(nb):
                pss = ps.tile([128, w], F32, tag="ps", name=f"pss{h}_{ci}")
                r = b - sb0
                # clip to the causally-valid column range, but keep the moving
                # free dim >= 256 (f32r drops to 1/4 rate below that)
                c0 = 128 * r if 0 < r <= (w - 256) // 128 else 0
                nc.tensor.matmul(
                    pss[:, c0:],
                    kT[h][:, 128 * b:128 * (b + 1)],
                    qT[h][:, s0 + c0:s0 + w],
                    start=True,
                    stop=True,
                )
                pt = ptp.tile([128, 512], F32R, tag="pt")
                c0 = 128 * r if r > 0 else 0
                nc.scalar.activation(
                    pt[:, c0:w], pss[:, c0:], mybir.ActivationFunctionType.Exp,
                    scale=SCALE,
                )
                if r >= 0:
                    nc.vector.tensor_mul(
                        pt[:, c0:w], pt[:, c0:w], msk[:, r, c0:w]
                    )
                stv = dict(start=(b == 0), stop=(b == nb - 1))
                nc.tensor.matmul(pd[:, c0:], onc[:], pt[:, c0:w], **stv)
                nc.tensor.matmul(
                    po[:, c0:], v_sb[:, b, 128 * h:128 * (h + 1)], pt[:, c0:w],
                    **stv
                )
            rec = work.tile([1, 512], F32, tag="rec", bufs=1)
            with nc.allow_low_precision("softmax denom recip"):
                nc.vector.reciprocal(rec[:, :w], pd[:])
            bc = work.tile([128, 512], F32, tag="bc")
            nc.gpsimd.partition_broadcast(bc[:, :w], rec[:, :w])
            nc.vector.tensor_mul(aT[h][:, s0:s0 + w], po[:], bc[:, :w])

        def stage_c(ci):
            s0, w = CHUNKS[ci]
            for m4 in range(w // 128):
                m = s0 // 128 + m4
                for e in range(NJ):
                    pf = ps.tile([128, 512], F32, tag="ps", name=f"pf{j}_{m4}")
                    for h2 in range(HPC):
                        nc.tensor.matmul(
                            pf[:],
                            aT[h2][:, 128 * m:128 * (m + 1)],
                            wo_s[:, h2, 512 * e:512 * (e + 1)],
                            start=(h2 == 0),
                            stop=(h2 == HPC - 1),
                        )
                    ot = outp.tile([128, 512], mybir.dt.float16, tag="ot")
                    if e % 2 == 0:
                        nc.vector.tensor_copy(ot[:], pf[:])
                    else:
                        nc.scalar.copy(ot[:], pf[:])
                    eng = nc.scalar if e % 2 == 0 else nc.sync
                    eng.dma_start(
                        out_d[128 * m:128 * (m + 1), 512 * e:512 * (e + 1)],
                        ot[:],
                    )

        for ci in range(len(CHUNKS)):
            if "A" in stages:
                stage_a(ci)
            if "B" in stages:
                for h in range(HPC):
                    stage_b(h, ci)
            if "C" in stages:
                stage_c(ci)

    nc.compile()
    return nc


def _host_inputs(x, Wq, Wk, Wv, Wo):
    x = np.asarray(x, dtype=np.float32).reshape(S, D)
    xt = np.ascontiguousarray(x.T)

    half = DH // 2
    inv_freq = (
        ROT_FACTOR
        / (ROPE_BASE ** (np.arange(0, half, dtype=np.float32) * 2.0 / DH))
    ).astype(np.float32)
    sgn = np.where(np.arange(DH) < half, -1.0, 1.0).astype(np.float32)[:, None]
    fd = np.concatenate([inv_freq, inv_freq]).astype(np.float32)[:, None]  # [128,1]
    u = np.arange(512, dtype=np.float32)[None, :]
    jj = (512.0 * np.arange(NJ, dtype=np.float32))[None, :]
    ropecu = np.cos(fd * u).astype(np.float32)            # [128, 512]
    ropesu = (sgn * np.sin(fd * u)).astype(np.float32)
    ropec512 = np.cos(fd * jj).astype(np.float32)         # [128, NJ]
    ropes512 = (sgn * np.sin(fd * jj)).astype(np.float32)

    onescol = np.ones((128, 1), dtype=np.float32)

    Wq = np.asarray(Wq, dtype=np.float32)
    Wk = np.asarray(Wk, dtype=np.float32)
    Wv = np.asarray(Wv, dtype=np.float32)
    Wo = np.asarray(Wo, dtype=np.float32)

    in_maps = []
    for c in range(NCORES):
        sl = slice(CW * c, CW * (c + 1))
        in_maps.append(
            {
                "xt": xt,
                "wq": np.ascontiguousarray(Wq[:, sl]),
                "wk": np.ascontiguousarray(Wk[:, sl]),
                "wv": np.ascontiguousarray(Wv[:, sl]),
                "wo": np.ascontiguousarray(Wo[sl, :]),
                "ropecu": ropecu,
                "ropesu": ropesu,
                "ropec512": ropec512,
                "ropes512": ropes512,
                "onescol": onescol,
            }
        )
    return in_maps


_NC_CACHE = None


def kernel(x, Wq, Wk, Wv, Wo):
    global _NC_CACHE
    if _NC_CACHE is None:
        _NC_CACHE = _build_nc()
    in_maps = _host_inputs(x, Wq, Wk, Wv, Wo)
    res = run_bass_kernel_spmd(_NC_CACHE, in_maps, core_ids=list(range(NCORES)))
    out = np.zeros((S, D), dtype=np.float32)
    for r in res.results:
        out += r["out"].astype(np.float32)
    return out.reshape(1, S, D)



# revision 10
# speedup vs baseline: 1.0621x; 1.0621x over previous
"""PhiHarmonicAttention (B=1, S=2048, D=2048, H=16, Dh=128) on 8 Trainium2 cores.

Sharding: tensor-parallel over heads - 2 heads per core.
  - Wq/Wk/Wv column-sliced (256 cols per core), Wo row-sliced (256 rows).
  - Host sums the 8 partial outputs (TP row-parallel reduction).

v2: bf16 operands everywhere (matmul accum stays fp32 in PSUM; rel-err budget
2e-2 >> bf16 noise ~5e-4), exact causal clipping, softmax denominator
accumulated on DVE (no per-block ones-matmuls on PE), and software-pipelined
emission: per 512-query chunk ci, PE work for B(ci) attention is interleaved
with A(ci+1) projections and C(ci-1) output-projection units so PE never waits
on the exp (ACT) chain.  PSUM static plan: 4 banks qkv-projection accumulators,
3 banks scores/output-proj rotation, 1 bank attention-output accumulator.
"""
import numpy as np
from contextlib import ExitStack, nullcontext

import concourse.bass as bass
import concourse.tile as tile
from concourse import bacc, mybir
from concourse.bass_utils import run_bass_kernel_spmd

S = 2048
D = 2048
H = 16
DH = 128
NCORES = 8
HPC = H // NCORES          # heads per core = 2
CW = HPC * DH              # weight col-slice per core = 256
NO = D // 128              # contraction chunks = 16
NJ = S // 512              # rope table chunks = 4
NB = S // 128              # 128-wide seq blocks = 16
NCH = S // 512             # 512-wide query chunks = 4
SCALE = float(1.0 / np.sqrt(np.float32(DH)))

ROT_FACTOR = (1.0 + 5.0 ** 0.5) / 2.0 - 1.0
ROPE_BASE = 10000.0

F32 = mybir.dt.float32
F32R = mybir.dt.float32r
BF16 = mybir.dt.bfloat16
F16 = mybir.dt.float16


def _build_nc(reps=1, stages="ABC"):
    nc = bacc.Bacc("TRN2", target_bir_lowering=False, debug=False, num_devices=NCORES)

    xt_d = nc.dram_tensor("xt", [D, S], BF16, kind="ExternalInput").ap()
    wq_d = nc.dram_tensor("wq", [D, CW], BF16, kind="ExternalInput").ap()
    wk_d = nc.dram_tensor("wk", [D, CW], BF16, kind="ExternalInput").ap()
    wv_d = nc.dram_tensor("wv", [D, CW], BF16, kind="ExternalInput").ap()
    wo_d = nc.dram_tensor("wo", [CW, D], BF16, kind="ExternalInput").ap()
    rcu_d = nc.dram_tensor("ropecu", [DH, 512], F32, kind="ExternalInput").ap()
    rsu_d = nc.dram_tensor("ropesu", [DH, 512], F32, kind="ExternalInput").ap()
    rc512_d = nc.dram_tensor("ropec512", [DH, NJ], F32, kind="ExternalInput").ap()
    rs512_d = nc.dram_tensor("ropes512", [DH, NJ], F32, kind="ExternalInput").ap()
    onc_d = nc.dram_tensor("onescol", [128, 1], F32R, kind="ExternalInput").ap()
    out_d = nc.dram_tensor("out", [S, D], F16, kind="ExternalOutput").ap()

    with ExitStack() as ctx:
        tc = ctx.enter_context(tile.TileContext(nc))
        consts = ctx.enter_context(tc.tile_pool(name="consts", bufs=1))
        persist = ctx.enter_context(tc.tile_pool(name="persist", bufs=1))
        xw = ctx.enter_context(tc.tile_pool(name="xw", bufs=20))
        ptp = ctx.enter_context(tc.tile_pool(name="ptp", bufs=6))
        dkp = ctx.enter_context(tc.tile_pool(name="dkp", bufs=2))
        work = ctx.enter_context(tc.tile_pool(name="work", bufs=2))
        recp = ctx.enter_context(tc.tile_pool(name="recp", bufs=2))
        outp = ctx.enter_context(tc.tile_pool(name="outp", bufs=2))
        psQ = ctx.enter_context(tc.tile_pool(name="psQ", bufs=4, space="PSUM"))
        psB = ctx.enter_context(tc.tile_pool(name="psB", bufs=3, space="PSUM"))
        psO = ctx.enter_context(tc.tile_pool(name="psO", bufs=1, space="PSUM"))

        # ---- constants ----
        wq_s = consts.tile([128, NO, CW], BF16, tag="wq")
        wk_s = consts.tile([128, NO, CW], BF16, tag="wk")
        wv_s = consts.tile([128, NO, CW], BF16, tag="wv")
        wo_s = consts.tile([128, HPC, D], BF16, tag="wo")
        rc = consts.tile([DH, S], F32, tag="rc")
        rs = consts.tile([DH, S], F32, tag="rs")
        tri = consts.tile([128, 128], BF16, tag="tri")
        onc = consts.tile([128, 1], F32R, tag="onc")
        rcu = consts.tile([DH, 512], F32, tag="rcu")
        rsu = consts.tile([DH, 512], F32, tag="rsu")
        rc512 = consts.tile([DH, NJ], F32, tag="rc512")
        rs512 = consts.tile([DH, NJ], F32, tag="rs512")

        nc.sync.dma_start(rcu[:], rcu_d)
        nc.sync.dma_start(rsu[:], rsu_d)
        nc.sync.dma_start(rc512[:], rc512_d)
        nc.sync.dma_start(rs512[:], rs512_d)
        nc.sync.dma_start(wq_s[:], wq_d.rearrange("(o p) n -> p o n", p=128))
        nc.sync.dma_start(wv_s[:], wv_d.rearrange("(o p) n -> p o n", p=128))
        nc.sync.dma_start(wk_s[:], wk_d.rearrange("(o p) n -> p o n", p=128))
        nc.sync.dma_start(onc[:], onc_d)
        nc.sync.dma_start(wo_s[:], wo_d.rearrange("(h p) n -> p h n", p=128))

        # rc/rs = full [128, 2048] rope tables via angle-addition from the
        # 512-wide unit tables (cos/sin(u) x cos/sin(512j)).
        for j in range(NJ):
            sl = slice(512 * j, 512 * (j + 1))
            tm = work.tile([128, 512], F32, tag="t1", name=f"tm{j}")
            nc.vector.tensor_scalar_mul(tm[:], rsu[:], rs512[:, j:j + 1])
            nc.vector.scalar_tensor_tensor(
                rc[:, sl], rcu[:], rc512[:, j:j + 1], tm[:],
                mybir.AluOpType.mult, mybir.AluOpType.subtract,
            )
            tm2 = work.tile([128, 512], F32, tag="tsw", name=f"tm2{j}")
            nc.vector.tensor_scalar_mul(tm2[:], rcu[:], rs512[:, j:j + 1])
            nc.vector.scalar_tensor_tensor(
                rs[:, sl], rsu[:], rc512[:, j:j + 1], tm2[:],
                mybir.AluOpType.mult, mybir.AluOpType.add,
            )
        # tri[p, c] = 1 if c >= p else 0  (within-block causal triangle)
        iot = work.tile([128, 512], F32, tag="t1", name="iot")
        nc.gpsimd.iota(
            iot[:, :128], pattern=[[1, 128]], base=0, channel_multiplier=-1,
            allow_small_or_imprecise_dtypes=True,
        )
        nc.vector.tensor_scalar(
            tri[:], iot[:, :128], 0.0, None, mybir.AluOpType.is_ge,
        )

        # ---- persistent per-head tensors ----
        qT = [persist.tile([DH, S], BF16, tag=f"qT{h}", name=f"qT{h}")
              for h in range(HPC)]
        kT = [persist.tile([DH, S], BF16, tag=f"kT{h}", name=f"kT{h}")
              for h in range(HPC)]
        v_sb = persist.tile([128, NB, CW], BF16, tag="v")
        aT = [persist.tile([DH, S], BF16, tag=f"aT{h}", name=f"aT{h}")
              for h in range(HPC)]

        xt_tiles = {}
        pqd, pkd, pvd = {}, {}, {}

        def xt_dma(ci):
            s0 = 512 * ci
            for o2 in range(NO // 2):
                t = xw.tile([128, 2, 512], BF16, tag="xt", name=f"xt{ci}_{o2}")
                nc.sync.dma_start(
                    t[:],
                    xt_d[256 * o2:256 * (o2 + 1), s0:s0 + 512].rearrange(
                        "(a p) n -> p a n", p=128
                    ),
                )
                xt_tiles[(ci, 2 * o2)] = t[:, 0, :]
                xt_tiles[(ci, 2 * o2 + 1)] = t[:, 1, :]

        def rope_emit(psrc, dstT, s0):
            sl = slice(s0, s0 + 512)
            t1 = work.tile([128, 512], F32, tag="t1")
            nc.vector.tensor_mul(t1[:], psrc[:], rc[:, sl])
            tsw = work.tile([128, 512], F32, tag="tsw")
            nc.vector.tensor_mul(tsw[0:64, :], psrc[64:128, :], rs[0:64, sl])
            nc.vector.tensor_mul(tsw[64:128, :], psrc[0:64, :], rs[64:128, sl])
            nc.vector.tensor_add(dstT[:, sl], t1[:], tsw[:])

        def a_q_unit(ci, o):
            def f():
                if o == 0:
                    for h in range(HPC):
                        pqd[(ci, h)] = psQ.tile(
                            [128, 512], F32, tag="ps", name=f"pq{ci}_{h}"
                        )
                st = dict(start=(o == 0), stop=(o == NO - 1))
                xt_t = xt_tiles[(ci, o)]
                for h in range(HPC):
                    nc.tensor.matmul(
                        pqd[(ci, h)][:], wq_s[:, o, 128 * h:128 * (h + 1)],
                        xt_t, **st
                    )
                if o == NO - 1:
                    for h in range(HPC):
                        rope_emit(pqd[(ci, h)], qT[h], 512 * ci)
            return f

        def a_k_unit(ci, o):
            def f():
                if o == 0:
                    for h in range(HPC):
                        pkd[(ci, h)] = psQ.tile(
                            [128, 512], F32, tag="ps", name=f"pk{ci}_{h}"
                        )
                st = dict(start=(o == 0), stop=(o == NO - 1))
                xt_t = xt_tiles[(ci, o)]
                for h in range(HPC):
                    nc.tensor.matmul(
                        pkd[(ci, h)][:], wk_s[:, o, 128 * h:128 * (h + 1)],
                        xt_t, **st
                    )
                if o == NO - 1:
                    for h in range(HPC):
                        rope_emit(pkd[(ci, h)], kT[h], 512 * ci)
            return f

        def a_v_unit(ci, o):
            def f():
                if o == 0:
                    for i in range(4):
                        pvd[(ci, i)] = psQ.tile(
                            [128, 256], F32, tag="ps", name=f"pv{ci}_{i}"
                        )
                st = dict(start=(o == 0), stop=(o == NO - 1))
                xt_t = xt_tiles[(ci, o)]
                for m4 in range(4):
                    nc.tensor.matmul(
                        pvd[(ci, m4)][:],
                        xt_t[:, 128 * m4:128 * (m4 + 1)],
                        wv_s[:, o, :], **st
                    )
                if o == NO - 1:
                    for i in range(4):
                        nc.scalar.copy(
                            v_sb[:, 4 * ci + i, :], pvd[(ci, i)][:]
                        )
            return f

        def a_stream(ci):
            return (
                [a_q_unit(ci, o) for o in range(NO)]
                + [a_v_unit(ci, o) for o in range(NO)]
                + [a_k_unit(ci, o) for o in range(NO)]
            )

        def b_stream(ci):
            s0 = 512 * ci
            sb0 = 4 * ci
            nb = 4 * (ci + 1)
            units = []
            for h in range(HPC):
                st = {}

                def mk_pss(b, h=h, st=st):
                    c0 = 128 * (b - sb0) if b >= sb0 else 0
                    t = psB.tile([128, 512], F32, tag="ps",
                                 name=f"pss{ci}_{h}_{b}")
                    st[("pss", b)] = (t, c0)
                    nc.tensor.matmul(
                        t[:, c0:], kT[h][:, 128 * b:128 * (b + 1)],
                        qT[h][:, s0 + c0:s0 + 512], start=True, stop=True,
                    )

                def mk_post(b, h=h, st=st):
                    t, c0 = st[("pss", b)]
                    pt = ptp.tile([128, 512], BF16, tag="pt",
                                  name=f"pt{ci}_{h}_{b}")
                    nc.scalar.activation(
                        pt[:, c0:], t[:, c0:],
                        mybir.ActivationFunctionType.Exp, scale=SCALE,
                    )
                    if b >= sb0:
                        nc.vector.tensor_mul(
                            pt[:, c0:c0 + 128], pt[:, c0:c0 + 128], tri[:]
                        )
                    # denominator: two parity-split accumulation chains
                    # (DVE for odd blocks, GpSimd for even) to spread load.
                    par = b % 2
                    acc = st["de"] if par == 0 else st["do"]
                    eng = nc.gpsimd if par == 0 else nc.vector
                    if b == par:
                        if c0 > 0:
                            nc.vector.memzero(pt[:, :c0])
                        eng.tensor_copy(acc[:], pt[:])
                    else:
                        eng.tensor_add(acc[:, c0:], acc[:, c0:], pt[:, c0:])
                    nc.tensor.matmul(
                        st["po"][:, c0:], v_sb[:, b, 128 * h:128 * (h + 1)],
                        pt[:, c0:], start=(b == 0), stop=(b == nb - 1),
                    )

                def u0(h=h, st=st, mps=mk_pss):
                    st["po"] = psO.tile([128, 512], F32, tag="po",
                                        name=f"po{ci}_{h}")
                    st["de"] = dkp.tile([128, 512], F32R, tag="de",
                                        name=f"de{ci}_{h}")
                    st["do"] = dkp.tile([128, 512], F32R, tag="do",
                                        name=f"do{ci}_{h}")
                    mps(0)
                    mps(1)

                def ub(b, h=h, st=st, mps=mk_pss, mpo=mk_post):
                    if b + 2 < nb:
                        mps(b + 2)
                    mpo(b)

                def ufin(h=h, st=st):
                    pdf = psB.tile([128, 512], F32, tag="ps",
                                   name=f"pdf{ci}_{h}")
                    nc.tensor.matmul(pdf[0:1, :], onc[:], st["de"][:],
                                     start=True, stop=False)
                    nc.tensor.matmul(pdf[0:1, :], onc[:], st["do"][:],
                                     start=False, stop=True)
                    rec = recp.tile([1, 512], F32, tag="rec",
                                    name=f"rec{ci}_{h}")
                    with nc.allow_low_precision("softmax denom recip"):
                        nc.vector.reciprocal(rec[:], pdf[0:1, :])
                    bc = recp.tile([128, 512], F32, tag="bc",
                                   name=f"bc{ci}_{h}")
                    nc.gpsimd.partition_broadcast(bc[:], rec[:])
                    nc.vector.tensor_mul(
                        aT[h][:, s0:s0 + 512], st["po"][:], bc[:]
                    )

                units.append(u0)
                units.extend(
                    (lambda b=b, f=ub: f(b)) for b in range(nb)
                )
                units.append(ufin)
            return units

        def c_stream(ci):
            otd = {}
            units = []

            def unit(m4, e):
                m = 4 * ci + m4
                if e == 0:
                    otd[m4] = outp.tile([128, D], F16, tag="ot",
                                        name=f"ot{ci}_{m4}")
                pf = psB.tile([128, 512], F32, tag="ps",
                              name=f"pf{ci}_{m4}_{e}")
                for h2 in range(HPC):
                    nc.tensor.matmul(
                        pf[:], aT[h2][:, 128 * m:128 * (m + 1)],
                        wo_s[:, h2, 512 * e:512 * (e + 1)],
                        start=(h2 == 0), stop=(h2 == HPC - 1),
                    )
                if e % 2 == 0:
                    nc.vector.tensor_copy(otd[m4][:, 512 * e:512 * (e + 1)],
                                          pf[:])
                else:
                    nc.scalar.copy(otd[m4][:, 512 * e:512 * (e + 1)], pf[:])
                if e == 3:
                    nc.gpsimd.dma_start(out_d[128 * m:128 * (m + 1), :],
                                        otd[m4][:])

            for m4 in range(4):
                for e in range(4):
                    units.append(lambda m4=m4, e=e: unit(m4, e))
            return units

        def run(units):
            for u in units:
                u()

        def interleave(streams):
            items = []
            for si, s in enumerate(streams):
                n = len(s)
                for j, u in enumerate(s):
                    items.append(((j + 0.5) / n, si, j, u))
            items.sort(key=lambda t: (t[0], t[1], t[2]))
            for it in items:
                it[3]()

        # ---- software-pipelined emission ----
        xt_dma(0)
        run(a_stream(0))
        xt_dma(1)
        interleave([b_stream(0), a_stream(1)])
        xt_dma(2)
        interleave([b_stream(1), a_stream(2), c_stream(0)])
        xt_dma(3)
        interleave([b_stream(2), a_stream(3), c_stream(1)])
        interleave([b_stream(3), c_stream(2)])
        run(c_stream(3))

    nc.compile()
    return nc


def _host_inputs(x, Wq, Wk, Wv, Wo):
    import ml_dtypes
    bf16 = ml_dtypes.bfloat16

    x = np.asarray(x, dtype=np.float32).reshape(S, D)
    xt = np.ascontiguousarray(x.T).astype(bf16)

    half = DH // 2
    inv_freq = (
        ROT_FACTOR
        / (ROPE_BASE ** (np.arange(0, half, dtype=np.float32) * 2.0 / DH))
    ).astype(np.float32)
    sgn = np.where(np.arange(DH) < half, -1.0, 1.0).astype(np.float32)[:, None]
    fd = np.concatenate([inv_freq, inv_freq]).astype(np.float32)[:, None]
    u = np.arange(512, dtype=np.float32)[None, :]
    jj = (512.0 * np.arange(NJ, dtype=np.float32))[None, :]
    ropecu = np.cos(fd * u).astype(np.float32)
    ropesu = (sgn * np.sin(fd * u)).astype(np.float32)
    ropec512 = np.cos(fd * jj).astype(np.float32)
    ropes512 = (sgn * np.sin(fd * jj)).astype(np.float32)

    onescol = np.ones((128, 1), dtype=np.float32)

    Wq = np.asarray(Wq, dtype=np.float32)
    Wk = np.asarray(Wk, dtype=np.float32)
    Wv = np.asarray(Wv, dtype=np.float32)
    Wo = np.asarray(Wo, dtype=np.float32)

    in_maps = []
    for c in range(NCORES):
        sl = slice(CW * c, CW * (c + 1))
        in_maps.append(
            {
                "xt": xt,
                "wq": np.ascontiguousarray(Wq[:, sl]).astype(bf16),
                "wk": np.ascontiguousarray(Wk[:, sl]).astype(bf16),
                "wv": np.ascontiguousarray(Wv[:, sl]).astype(bf16),
                "wo": np.ascontiguousarray(Wo[sl, :]).astype(bf16),
                "ropecu": ropecu,
                "ropesu": ropesu,
                "ropec512": ropec512,
                "ropes512": ropes512,
                "onescol": onescol,
            }
        )
    return in_maps


_NC_CACHE = None


def kernel(x, Wq, Wk, Wv, Wo):
    global _NC_CACHE
    if _NC_CACHE is None:
        _NC_CACHE = _build_nc()
    in_maps = _host_inputs(x, Wq, Wk, Wv, Wo)
    res = run_bass_kernel_spmd(_NC_CACHE, in_maps, core_ids=list(range(NCORES)))
    out = np.zeros((S, D), dtype=np.float32)
    for r in res.results:
        out += r["out"].astype(np.float32)
    return out.reshape(1, S, D)
